# revision 1
# baseline (speedup 1.0000x reference)
"""Trainium2 Bass kernel for nn_DepthWiseSepConv (depthwise 5x5 + BN+hardswish
+ pointwise 1x1 + squeeze-excite gating + BN), data-parallel over batch on
8 NeuronCores.

Self-contained: hardcodes all shapes from the problem spec.

Per-core layout strategy (B_loc = 8 images per core):
  - Depthwise conv: partitions = (4 channels x 28 rows of H). For each of the
    5 kernel columns dx, one matmul with a host-built block-diagonal Toeplitz
    matrix (contracting h_in -> h_out) against x shifted by dx along W (zero
    padded in SBUF). The 5 matmuls accumulate in PSUM.
  - BN1 + hardswish fused: ACT relu(psum*s1 + t1+3), then
    act = (a-3) * min(a/6, 1).
  - Two TensorE transpose stages to reach channel-major [c, (b,h,w)] layout
    for the pointwise conv.
  - SE: DVE free-dim reduce for the mean, two small matmuls, hardswish.
  - Pointwise conv: [120c x 120o] matmul tiles, N=392 (half an image),
    epilogue fuses +pw_b, *g (SE gate), BN2.
"""

import sys

sys.path.insert(0, "/opt/trn_rl_repo")

import numpy as np
import ml_dtypes

import concourse.bass as bass
import concourse.mybir as mybir
import concourse.tile as tile
from concourse import bacc
from concourse.bass_utils import run_bass_kernel_spmd
from concourse.masks import make_identity

# ---------------------------------------------------------------- constants
N_CORES = 8
B, C, H, W = 64, 240, 28, 28
NB = B // N_CORES          # images per core
KK = 5                      # depthwise kernel size
G = C // 4                  # channel groups of 4 -> 60
R = 60                      # SE reduction dim
Cout = 240
HW = H * W                  # 784
EPS = 1e-5
WP = 36                     # padded W in SBUF x tiles (w in [-2, 34))
PIX = NB * HW               # 6272 pixels per core

CFG = {
    # dtype of DW + PW matmul operands: "float32" | "float32r" | "bfloat16"
    "mm_dt": "float32r",
    # dtype of activation storage / transposes: "float32" | "bfloat16"
    "act_dt": "float32",
    # DW psum free width: 28 (exact) or 32 (padded, helps float32r)
    "wout": 32,
    # pack DW Toeplitz as 4x[32,32] tile_position blocks (3x less weight DMA)
    "packed": False,
    # DRAM storage dtype of toep; float16 halves DMA, cast to f32 in flight
    "toep_store": "float16",
    # transfer only the 4 diagonal 28x28 blocks of each Toeplitz (4x less
    # DMA); persistent pre-zeroed SBUF buffers, manual 3-way rotation
    "toep_compact": False,
    # debug: emit only a prefix of the phases ("a"|"ab"|"abc"|"")
    "stop_after": "",
    # x DMA batching: groups loaded per DMA (1 or 2)
    "xbatch": 1,
    # rotation depth for x/toep persistent buffers
    "nrot": 4,
    # DW psum pool depth
    "dwbufs": 4,
}

_DT = {
    "float32": mybir.dt.float32,
    "float32r": mybir.dt.float32r,
    "bfloat16": mybir.dt.bfloat16,
}
_NPDT = {
    "float32": np.float32,
    "float32r": np.float32,
    "bfloat16": ml_dtypes.bfloat16,
}


def _f32v(ap):
    """View a float32r AP as plain float32 (for non-matmul readers)."""
    if ap.dtype == mybir.dt.float32r:
        return ap.bitcast(mybir.dt.float32)
    return ap


# ---------------------------------------------------------------- builder
_BUILD_CACHE = {}


def build_nc(cfg_key=None):
    cfg = dict(CFG)
    if cfg_key is not None:
        cfg.update(cfg_key)
    key = tuple(sorted(cfg.items()))
    if key in _BUILD_CACHE:
        return _BUILD_CACHE[key]

    mm_dt = _DT[cfg["mm_dt"]]
    act_dt = _DT[cfg["act_dt"]]
    WOUT = cfg["wout"]
    dw_r = cfg["mm_dt"] == "float32r"
    pw_r = dw_r and cfg["act_dt"] == "float32"
    # dtype of the PW matmul operands (weights + transposed activations)
    pw_dt = mybir.dt.float32r if pw_r else act_dt

    nc = bacc.Bacc("TRN2", target_bir_lowering=False, debug=False,
                   num_devices=N_CORES)

    packed = cfg["packed"]
    DWP = 128 if packed else 112     # DW partition count
    HB = 32 if packed else H         # per-channel partition block
    toep_st = mm_dt
    if cfg["toep_store"] == "float16" and cfg["mm_dt"] != "bfloat16":
        toep_st = mybir.dt.float16

    f32 = mybir.dt.float32
    x_dram_dt = f32 if cfg["mm_dt"] == "bfloat16" else mm_dt
    x_p = nc.declare_dram_parameter("x", [NB, C, H, W], x_dram_dt,
                                    isOutput=False)
    if packed:
        toep_p = nc.declare_dram_parameter("toep", [G, 4, 32, KK, 32], toep_st,
                                           isOutput=False)
    elif cfg["toep_compact"]:
        toep_p = nc.declare_dram_parameter("toep", [G, 4, H, KK, H], toep_st,
                                           isOutput=False)
    else:
        toep_p = nc.declare_dram_parameter("toep", [G, 112, KK, 112], toep_st,
                                           isOutput=False)
    bn1s_p = nc.declare_dram_parameter("bn1s", [DWP, G], f32, isOutput=False)
    bn1b_p = nc.declare_dram_parameter("bn1b", [DWP, G], f32, isOutput=False)
    pwl_p = nc.declare_dram_parameter("pwl", [2, 120, 2, 120], pw_dt,
                                      isOutput=False)
    se1l_p = nc.declare_dram_parameter("se1l", [2, 120, R], f32, isOutput=False)
    se1b_p = nc.declare_dram_parameter("se1b", [R, 1], f32, isOutput=False)
    se2l_p = nc.declare_dram_parameter("se2l", [R, 2, 120], f32, isOutput=False)
    se2b3_p = nc.declare_dram_parameter("se2b3", [120, 2], f32, isOutput=False)
    bn2s_p = nc.declare_dram_parameter("bn2s", [120, 2], f32, isOutput=False)
    bn2sb_p = nc.declare_dram_parameter("bn2sb", [120, 2], f32, isOutput=False)
    bn2t_p = nc.declare_dram_parameter("bn2t", [120, 2], f32, isOutput=False)
    # zero-fill source (walrus rejects Memset on float32r tiles)
    zeros_p = nc.declare_dram_parameter("zeros", [128, 640], mm_dt,
                                        isOutput=False)
    y_p = nc.declare_dram_parameter("y", [NB, Cout, H, W], f32, isOutput=True)

    AL = mybir.AluOpType

    with tile.TileContext(nc) as tc:
        cst = tc.alloc_tile_pool(name="cst", bufs=1)
        pers = tc.alloc_tile_pool(name="pers", bufs=1)

        # ---- constants in SBUF
        bn1s_sb = cst.tile([DWP, G], f32)
        nc.sync.dma_start(bn1s_sb[:], bn1s_p[:])
        bn1b_sb = cst.tile([DWP, G], f32)
        nc.sync.dma_start(bn1b_sb[:], bn1b_p[:])
        pwl_sb = cst.tile([120, 2, 2, 120], pw_dt)  # [K=c, kc, mo, M=o]
        nc.sync.dma_start(pwl_sb[:], pwl_p[:].rearrange("kc k mo m -> k kc mo m"))
        se1l_sb = cst.tile([120, 2, R], f32)
        nc.sync.dma_start(se1l_sb[:], se1l_p[:].rearrange("kc k r -> k kc r"))
        se1b_sb = cst.tile([R, 1], f32)
        nc.sync.dma_start(se1b_sb[:], se1b_p[:])
        se2l_sb = cst.tile([R, 2, 120], f32)
        nc.sync.dma_start(se2l_sb[:], se2l_p[:])
        se2b3_sb = cst.tile([120, 2], f32)
        nc.sync.dma_start(se2b3_sb[:], se2b3_p[:])
        bn2s_sb = cst.tile([120, 2], f32)
        nc.sync.dma_start(bn2s_sb[:], bn2s_p[:])
        bn2sb_sb = cst.tile([120, 2], f32)
        nc.sync.dma_start(bn2sb_sb[:], bn2sb_p[:])
        bn2t_sb = cst.tile([120, 2], f32)
        nc.sync.dma_start(bn2t_sb[:], bn2t_p[:])

        ident = cst.tile([128, 128], act_dt)
        make_identity(nc, ident[:])

        # persistent activation buffers
        # ActT[ch]: [(b4,w28)=112, q, (g_local, c4, h) = 30*112]
        ActT = [pers.tile([112, 2, 30 * 112], act_dt, name=f"actt_{ch}")
                for ch in range(2)]
        # PWrhs[ch]: [c=120, (b, h, w) = 6272]
        PWrhs = [pers.tile([120, PIX], pw_dt, name=f"pwrhs_{ch}")
                 for ch in range(2)]
        g_sb = [pers.tile([120, NB], f32, name=f"gate_{mo}") for mo in range(2)]

        # x rearranged for DW rhs: dims (c4, h, g, b, w)
        x_r = x_p[:].rearrange("b (g c) h w -> c h g b w", c=4)
        # merged (c h) partition form for the unpacked single-DMA load
        x_rm = x_p[:].rearrange("b (g c) h w -> (c h) g b w", c=4)

        # persistent DW input buffers, manual 3-way rotation: zero padding is
        # written once, per-group DMAs only overwrite the payload regions
        NROT = cfg["nrot"]
        XB = cfg["xbatch"]
        x_bufs = [pers.tile([DWP, XB, NB, WP], mm_dt, name=f"x_rot{i}")
                  for i in range(NROT)]
        zx = zeros_p[:, :XB * NB * WP].rearrange(
            "p (xb nb wp) -> p xb nb wp", xb=XB, nb=NB)
        for xb in x_bufs:
            nc.sync.dma_start(xb[:], zx[:DWP])
        toep_bufs = None
        if cfg["toep_compact"] and not packed:
            toep_bufs = [pers.tile([112, KK, 112], mm_dt, name=f"tp_rot{i}")
                         for i in range(NROT)]
            zt = zeros_p[:, :KK * 112].rearrange("p (k m) -> p k m", k=KK)
            for tb in toep_bufs:
                nc.sync.dma_start(tb[:], zt[:112])

        # ================= Phase A: depthwise + BN1 + hardswish + T1
        with tc.tile_pool(name="pa", bufs=3) as pa, \
             tc.tile_pool(name="pa2", bufs=4) as pa2, \
             tc.tile_pool(name="dwps", bufs=cfg["dwbufs"], space="PSUM") as dwps, \
             tc.tile_pool(name="t1ps", bufs=3, space="PSUM") as t1ps:
            for g in range(G):
                gb, gi = divmod(g, XB)
                x_gb = x_bufs[gb % NROT]
                x_g = x_gb[:, gi]
                x_dma = (nc.gpsimd.dma_start
                         if cfg["mm_dt"] == "bfloat16" else nc.sync.dma_start)
                if gi == 0:
                    if packed:
                        # SBUF APs have a single partition dim -> one DMA
                        # per 32-row channel block
                        for ci in range(4):
                            x_dma(x_gb[32 * ci:32 * ci + H, 0, :, 2:2 + W],
                                  x_r[ci, :, g])
                    elif XB == 1:
                        x_dma(x_gb[:, 0, :, 2:2 + W], x_rm[:, g])
                    else:
                        x_dma(x_gb[:, :, :, 2:2 + W],
                              x_rm[:, g:g + XB])
                toep_dma = (nc.gpsimd.dma_start if toep_st != mm_dt
                            else nc.sync.dma_start)
                if packed:
                    toep_g = pa.tile([128, KK, 32], mm_dt, tag="toep_g")
                    toep_dma(toep_g[:],
                             toep_p[g].rearrange("c e dx m -> (c e) dx m"))
                elif cfg["toep_compact"]:
                    toep_g = toep_bufs[g % NROT]
                    for ci in range(4):
                        toep_dma(
                            toep_g[H * ci:H * ci + H, :, H * ci:H * ci + H],
                            toep_p[g, ci])
                else:
                    toep_g = pa.tile([112, KK, 112], mm_dt, tag="toep_g")
                    toep_dma(toep_g[:], toep_p[g])

                ps = dwps.tile([DWP, NB, WOUT], f32, tag="dw")
                for dx in range(KK):
                    if packed:
                        # one accumulation group for the whole psum region:
                        # start clears has_written bank-wide; per-element
                        # has_written handles first-write-overwrite for the
                        # other 32-row blocks
                        for ci in range(4):
                            nc.tensor.matmul(
                                ps[32 * ci:32 * ci + 32],
                                toep_g[32 * ci:32 * ci + 32, dx, :],
                                x_g[32 * ci:32 * ci + 32, :, dx:dx + WOUT],
                                start=(dx == 0 and ci == 0),
                                stop=(dx == KK - 1 and ci == 3),
                                tile_position=(32 * ci, 32 * ci),
                            )
                    else:
                        nc.tensor.matmul(
                            ps[:],
                            toep_g[:, dx, :],
                            x_g[:, :, dx:dx + WOUT],
                            start=(dx == 0),
                            stop=(dx == KK - 1),
                        )

                a_g = pa2.tile([DWP, NB, WOUT], f32, tag="a_g")
                nc.scalar.activation(a_g[:], ps[:],
                                     mybir.ActivationFunctionType.Relu,
                                     bias=bn1b_sb[:, g:g + 1],
                                     scale=bn1s_sb[:, g:g + 1])
                a_v = a_g[:, :, 0:W]
                m_g = pa2.tile([DWP, NB, W], f32, tag="m_g")
                nc.gpsimd.tensor_scalar(m_g[:], a_v, 1.0 / 6.0, 1.0,
                                        AL.mult, AL.min)
                act_g = pa2.tile([DWP, NB, W], act_dt, tag="act_g")
                nc.vector.scalar_tensor_tensor(act_g[:], a_v, 3.0, m_g[:],
                                               AL.subtract, AL.mult)

                ch, gl = (0, g) if g < 30 else (1, g - 30)
                tp = t1ps.tile([112, 2, DWP], act_dt, tag="t1")
                for q in range(2):
                    nc.tensor.transpose(tp[:, q, :],
                                        act_g[:, 4 * q:4 * q + 4, :],
                                        ident[:DWP, :DWP])
                # select real (c4, h) columns out of each DWP block
                tp_sel = tp[:].rearrange("p q (c e) -> p q c e", c=4)[
                    :, :, :, 0:H]
                nc.scalar.copy(
                    ActT[ch][:, :, gl * 112:(gl + 1) * 112].rearrange(
                        "p q (c e) -> p q c e", c=4),
                    tp_sel)

        # ================= Phase B: T2 -> channel-major PWrhs
        with tc.tile_pool(name="t2ps", bufs=6, space="PSUM") as t2ps:
            for ch in range(2):
                for q in range(2):
                    src4 = ActT[ch][:].rearrange(
                        "p q (gl c e) -> p q gl c e", gl=30, c=4)
                    dst4 = PWrhs[ch][:].rearrange(
                        "p (b hh w) -> p b hh w", b=NB, hh=H)
                    for h0 in range(0, H, 4):
                        tp = t2ps.tile([120, 4, 112], act_dt, tag="t2")
                        for hi in range(4):
                            # 120 cols: (g_local str 112) x (c4 str 28), off h
                            nc.tensor.transpose(tp[:, hi, :],
                                                src4[:, q, :, :, h0 + hi],
                                                ident[:112, :112])
                        nc.vector.tensor_copy(
                            dst4[:, 4 * q:4 * q + 4, h0:h0 + 4, :],
                            tp[:].rearrange("p hh (b w) -> p b hh w", b=4))

        # ================= Phase C: squeeze-excite
        with tc.tile_pool(name="se", bufs=1) as sep, \
             tc.tile_pool(name="seps", bufs=2, space="PSUM") as seps:
            s_sb = [sep.tile([120, NB], f32, name=f"s_{ch}") for ch in range(2)]
            for ch in range(2):
                nc.vector.tensor_reduce(
                    s_sb[ch][:],
                    _f32v(PWrhs[ch][:]).rearrange("p (b f) -> p b f", b=NB),
                    mybir.AxisListType.X, AL.add)
            ps1 = seps.tile([R, NB], f32, tag="se1")
            for ch in range(2):
                nc.tensor.matmul(ps1[:], se1l_sb[:, ch, :], s_sb[ch][:],
                                 start=(ch == 0), stop=(ch == 1))
            h1 = sep.tile([R, NB], f32)
            nc.scalar.activation(h1[:], ps1[:],
                                 mybir.ActivationFunctionType.Relu,
                                 bias=se1b_sb[:, 0:1])
            for mo in range(2):
                ps2 = seps.tile([120, NB], f32, tag="se2")
                nc.tensor.matmul(ps2[:], se2l_sb[:, mo, :], h1[:],
                                 start=True, stop=True)
                a2 = sep.tile([120, NB], f32, name=f"a2_{mo}")
                nc.scalar.activation(a2[:], ps2[:],
                                     mybir.ActivationFunctionType.Relu,
                                     bias=se2b3_sb[:, mo:mo + 1])
                m2 = sep.tile([120, NB], f32, name=f"m2_{mo}")
                nc.vector.tensor_scalar(m2[:], a2[:], 1.0 / 6.0, 1.0,
                                        AL.mult, AL.min)
                nc.vector.scalar_tensor_tensor(g_sb[mo][:], a2[:], 3.0, m2[:],
                                               AL.subtract, AL.mult)

        # ================= Phase D: pointwise conv + gate + BN2 + output
        NT = 392  # half an image
        with tc.tile_pool(name="pd", bufs=6) as pd, \
             tc.tile_pool(name="pdps", bufs=4, space="PSUM") as pdps:
            for mo in range(2):
                for b in range(NB):
                    for nt in range(2):
                        off = b * HW + nt * NT
                        ps = pdps.tile([120, NT], f32, tag="pw")
                        for kc in range(2):
                            nc.tensor.matmul(
                                ps[:],
                                pwl_sb[:, kc, mo, :],
                                PWrhs[kc][:, off:off + NT],
                                start=(kc == 0), stop=(kc == 1))
                        e2 = pd.tile([120, NT], f32, tag="e2")
                        nc.scalar.activation(
                            e2[:], ps[:],
                            mybir.ActivationFunctionType.Identity,
                            bias=bn2sb_sb[:, mo:mo + 1],
                            scale=bn2s_sb[:, mo:mo + 1])
                        f_t = pd.tile([120, NT], f32, tag="f_t")
                        nc.vector.tensor_tensor(
                            f_t[:], e2[:],
                            g_sb[mo][:, b:b + 1].to_broadcast((120, NT)),
                            AL.mult)
                        o_t = pd.tile([120, NT], f32, tag="o_t")
                        nc.gpsimd.tensor_scalar(o_t[:], f_t[:],
                                                bn2t_sb[:, mo:mo + 1], None,
                                                AL.add)
                        y_ap = y_p[b, mo * 120:(mo + 1) * 120].rearrange(
                            "c h w -> c (h w)")[:, nt * NT:(nt + 1) * NT]
                        nc.sync.dma_start(y_ap, o_t[:])

        pers.release()
        cst.release()

    nc.compile()
    _BUILD_CACHE[key] = nc
    return nc


# ---------------------------------------------------------------- host prep
def prep_inputs(inputs, cfg_key=None):
    cfg = dict(CFG)
    if cfg_key is not None:
        cfg.update(cfg_key)
    mmnp = _NPDT[cfg["mm_dt"]]
    f32 = np.float32

    x = np.asarray(inputs["x"], f32)
    dw_w = np.asarray(inputs["dw_w"], f32)      # [C,1,5,5]
    dw_b = np.asarray(inputs["dw_b"], f32)
    bn1_g = np.asarray(inputs["bn1_g"], f32)
    bn1_b = np.asarray(inputs["bn1_b"], f32)
    bn1_m = np.asarray(inputs["bn1_m"], f32)
    bn1_v = np.asarray(inputs["bn1_v"], f32)
    pw_w = np.asarray(inputs["pw_w"], f32)      # [Cout, C]
    pw_b = np.asarray(inputs["pw_b"], f32)
    se_w1 = np.asarray(inputs["se_w1"], f32)    # [R, C]
    se_b1 = np.asarray(inputs["se_b1"], f32)
    se_w2 = np.asarray(inputs["se_w2"], f32)    # [Cout, R]
    se_b2 = np.asarray(inputs["se_b2"], f32)
    bn2_g = np.asarray(inputs["bn2_g"], f32)
    bn2_b = np.asarray(inputs["bn2_b"], f32)
    bn2_m = np.asarray(inputs["bn2_m"], f32)
    bn2_v = np.asarray(inputs["bn2_v"], f32)

    packed = cfg["packed"]
    HB = 32 if packed else H
    s1 = bn1_g / np.sqrt(bn1_v + EPS)
    t1 = s1 * (dw_b - bn1_m) + bn1_b

    def _pp(v):  # [C] -> [DWP, G] per-partition vector, zero-padded blocks
        a = np.zeros((G, 4, HB), f32)
        a[:, :, :H] = v.reshape(G, 4)[:, :, None]
        return np.ascontiguousarray(a.reshape(G, 4 * HB).T)

    bn1s = _pp(s1)
    bn1b = _pp(t1 + 3.0)

    # Toeplitz blockdiag: toep[g, ci*28+hin, dx, cj*28+hout]
    #   = dw_w[4g+ci, 0, hin-hout+2, dx] if ci==cj and |hin-hout|<=2
    hin = np.arange(H)[:, None]
    hout = np.arange(H)[None, :]
    D = hin - hout
    mask = np.abs(D) <= 2
    dyi = np.clip(D + 2, 0, 4)
    k = dw_w[:, 0]                                                # [C, 5, 5]
    # band[c, hin, hout, dx]
    band = np.where(mask[None, :, :, None], k[:, dyi, :], 0.0)    # [C,28,28,5]
    band_r = band.reshape(G, 4, H, H, KK)           # [g, ci, hin, hout, dx]
    if packed:
        # [G, 4, 32(hin), KK, 32(hout)] zero-padded per-channel blocks
        toep = np.zeros((G, 4, 32, KK, 32), f32)
        toep[:, :, :H, :, :H] = band_r.transpose(0, 1, 2, 4, 3)
    elif cfg["toep_compact"]:
        # [G, 4, hin, KK, hout] dense diagonal blocks only
        toep = np.ascontiguousarray(band_r.transpose(0, 1, 2, 4, 3))
    else:
        toep = np.zeros((G, 4, H, KK, 4, H), f32)
        for ci in range(4):
            # [g, hin, dx, hout]
            toep[:, ci, :, :, ci, :] = band_r[:, ci].transpose(0, 1, 3, 2)
        toep = toep.reshape(G, 112, KK, 112)
    if cfg["toep_store"] == "float16" and cfg["mm_dt"] != "bfloat16":
        toep = toep.astype(np.float16)
    else:
        toep = toep.astype(mmnp)

    actnp = _NPDT[cfg["act_dt"]]
    pwT = np.ascontiguousarray(pw_w.T)               # [C, Cout]
    pwl = np.zeros((2, 120, 2, 120), f32)
    for kc in range(2):
        for mo in range(2):
            pwl[kc, :, mo, :] = pwT[kc * 120:(kc + 1) * 120,
                                    mo * 120:(mo + 1) * 120]
    pwl = pwl.astype(actnp)

    se1l = np.ascontiguousarray((se_w1.T / HW).reshape(2, 120, R))
    se1b = se_b1.reshape(R, 1).copy()
    se2l = np.ascontiguousarray(se_w2.T.reshape(R, 2, 120))
    se2b3 = np.ascontiguousarray((se_b2 + 3.0).reshape(2, 120).T)
    s2 = bn2_g / np.sqrt(bn2_v + EPS)
    bn2s = np.ascontiguousarray(s2.reshape(2, 120).T)
    bn2sb = np.ascontiguousarray((s2 * pw_b).reshape(2, 120).T)
    bn2t = np.ascontiguousarray((bn2_b - bn2_m * s2).reshape(2, 120).T)

    shared = {
        "toep": toep, "bn1s": bn1s, "bn1b": bn1b, "pwl": pwl,
        "se1l": se1l.astype(f32), "se1b": se1b, "se2l": se2l.astype(f32),
        "se2b3": se2b3, "bn2s": bn2s, "bn2sb": bn2sb, "bn2t": bn2t,
        "zeros": np.zeros((128, 640), mmnp),
    }
    in_maps = []
    for i in range(N_CORES):
        m = dict(shared)
        m["x"] = np.ascontiguousarray(x[i * NB:(i + 1) * NB])
        in_maps.append(m)
    return in_maps


def kernel(**inputs):
    nc = build_nc()
    in_maps = prep_inputs(inputs)
    res = run_bass_kernel_spmd(nc, in_maps, list(range(N_CORES)))
    out = np.concatenate([res.results[i]["y"] for i in range(N_CORES)], axis=0)
    return out.astype(np.float32)



# revision 49
# speedup vs baseline: 2.3002x; 2.3002x over previous
"""Trainium2 Bass kernel for nn_DepthWiseSepConv (depthwise 5x5 + BN+hardswish
+ pointwise 1x1 + squeeze-excite gating + BN), data-parallel over batch on
8 NeuronCores.

Self-contained: hardcodes all shapes from the problem spec.

Per-core layout (B_loc = 8 images per core), all matmul operands fp16:

  - Depthwise conv, operand-swapped Toeplitz form: stationary lhsT = x
    [(2ch x 28h_in) + ones-row, (4b x 28w_out)], moving rhs = per-group
    block-diagonal Toeplitz [(2ch x 28h_in) + bias-row, (2ch x 28h_out)].
    Output lands as [(4b, w), (c2, h)] so NO transpose is needed before the
    pointwise stage's channel-major transpose.  BN1 scale is folded into the
    Toeplitz weights; BN1 bias (+3 for the hardswish trick) rides an extra
    ones-row of x (dx=0 slice only).  5 dx shifts accumulate in PSUM.
    Channel blocks live at partition bases 0 and 64 (tile_position rule).
  - Hardswish epilogue: ACT relu (PSUM->f16), DVE min, DVE (a-3)*m -> D.
  - One transpose stage (T2): [(4b,w), (g,c)@h] -> channel-major R tiles.
  - SE: per-image DVE reduces on R, two small matmuls, hardswish; gate and
    BN2 are folded into per-(image,half) ACT scale/bias vectors.
  - Pointwise: [120c x 120o] fp16 matmuls, N=392, single fused epilogue op.
"""

import sys

sys.path.insert(0, "/opt/trn_rl_repo")

import numpy as np

import concourse.bass as bass
import concourse.mybir as mybir
import concourse.tile as tile
from concourse import bacc
from concourse.bass_utils import run_bass_kernel_spmd
from concourse.masks import make_identity

# ---------------------------------------------------------------- constants
N_CORES = 8
B, C, H, W = 64, 240, 28, 28
NB = B // N_CORES           # images per core
KK = 5                      # depthwise kernel size
G = 60                      # groups of 4 channels
Cout = 240
R = 60                      # SE reduction dim
HW = H * W
EPS = 1e-5
NT = 392                    # pointwise free-dim tile (half an image)
XP = 121                    # arena partitions: block0 @0..57, block1 @64..121
NCB = 369                   # packed f32 const-blob columns

CFG = {
    "xchunks": (4, 8, 10, 12, 12, 14),  # group counts per x DMA chunk
    "tch": 4,               # groups per toeplitz rotation slot
    "trot": 4,              # toeplitz rotation depth
    "dwbufs": 5,            # DW psum pool depth
    "t2bufs": 3,            # T2 psum pool depth
    "copy_rr": "vvs",       # T2 copy engine pattern (v=DVE, s=ACT, g=Pool)
    "d_rr": "sv",          # phase D epilogue engine pattern
    "stt_eng": "v",         # hardswish (a-3)*m engine
    "red_rr": "vs",         # SE reduce engines (v=DVE, s=ACT accum)
    "look": 18,             # input DMA lookahead (groups)
    "pdbufs": 6,            # pointwise psum pool depth
}

F16 = mybir.dt.float16
F32 = mybir.dt.float32


# ---------------------------------------------------------------- builder
_BUILD_CACHE = {}


def build_nc(cfg_key=None):
    cfg = dict(CFG)
    if cfg_key is not None:
        cfg.update(cfg_key)
    key = tuple(sorted((k, str(v)) for k, v in cfg.items()))
    if key in _BUILD_CACHE:
        return _BUILD_CACHE[key]

    nc = bacc.Bacc("TRN2", target_bir_lowering=False, debug=False,
                   num_devices=N_CORES)

    xar_p = nc.declare_dram_parameter("xar", [XP, G, 2, 36, 4], F16,
                                      isOutput=False)
    tpar_p = nc.declare_dram_parameter("tpar", [XP, G, KK, 56], F16,
                                       isOutput=False)
    zer_p = nc.declare_dram_parameter("zer", [XP, cfg["tch"], KK, 56], F16,
                                      isOutput=False)
    pwl_p = nc.declare_dram_parameter("pwl", [120, 2, 2, 120], F16,
                                      isOutput=False)
    cblob_p = nc.declare_dram_parameter("cblob", [120, NCB], F32,
                                        isOutput=False)
    y_p = nc.declare_dram_parameter("y", [NB, Cout, H, W], F16, isOutput=True)

    AL = mybir.AluOpType
    AF = mybir.ActivationFunctionType

    ENG = {"v": nc.vector, "s": nc.scalar, "g": nc.gpsimd}

    with tile.TileContext(nc) as tc:
        cst = tc.alloc_tile_pool(name="cst", bufs=1)
        pers = tc.alloc_tile_pool(name="pers", bufs=1)

        # ---- persistent arenas (chunked input DMAs for pipelined start)
        xchunks = list(cfg["xchunks"])
        assert sum(xchunks) == G
        xstarts = [sum(xchunks[:i]) for i in range(len(xchunks))]
        x_ch = [pers.tile([XP, n, 2, 36, 4], F16, name=f"xch{i}")
                for i, n in enumerate(xchunks)]
        xmap = {}
        for i, (s, n) in enumerate(zip(xstarts, xchunks)):
            for j in range(n):
                xmap[s + j] = (i, j)

        # toeplitz rotation slots: [121, TCH, blk, 5, 56] where the matmul
        # rhs view [:, g, :, dx, :] is the block-diagonal [121, 112].  The
        # blk-major layout keeps each fill DMA's runs at 560B (no small-
        # transfer penalty).  Zero halves are written once per slot.
        TCH, TROT = cfg["tch"], cfg["trot"]
        NTC = G // TCH
        t_sl = [pers.tile([XP, TCH, 2, KK, 56], F16, name=f"tsl{i}")
                for i in range(TROT)]

        def zero_slot(i):
            sl = t_sl[i]
            nc.gpsimd.dma_start(sl[0:57, :, 1], zer_p[0:57])
            nc.gpsimd.dma_start(sl[64:121, :, 0], zer_p[64:121])

        def fill_toep(c):
            sl = t_sl[c % TROT]
            s = c * TCH
            nc.sync.dma_start(sl[0:64, :, 0], tpar_p[0:64, s:s + TCH])
            nc.sync.dma_start(sl[57:121, :, 1], tpar_p[57:121, s:s + TCH])

        def issue_x(i):
            nc.sync.dma_start(x_ch[i][:],
                              xar_p[:, xstarts[i]:xstarts[i] + xchunks[i]])

        fill_toep(0)
        issue_x(0)
        zero_slot(0)
        issue_x(1)
        for i in range(1, TROT):
            fill_toep(i)
            zero_slot(i)

        # ---- constants in SBUF (single packed f32 blob + f16 pwl)
        pwl_sb = cst.tile([120, 2, 2, 120], F16)       # [K=c, kc, mo, M=o]
        nc.sync.dma_start(pwl_sb[:], pwl_p[:])
        cblob = cst.tile([120, NCB], F32)
        nc.sync.dma_start(cblob[:], cblob_p[:])
        se1l_sb = cblob[:, 0:120].rearrange("p (kc r) -> p kc r", kc=2)
        se1b_sb = cblob[0:R, 120:121]
        se2l_sb = cblob[0:R, 121:361].rearrange("p (mo o) -> p mo o", mo=2)
        se2b3_sb = cblob[:, 361:363]
        bn2s_sb = cblob[:, 363:365]
        bn2sb_sb = cblob[:, 365:367]
        bn2t_sb = cblob[:, 367:369]

        ident = cst.tile([128, 128], F16)
        make_identity(nc, ident[:])

        # remaining input DMAs are emitted inside the group loop, ordered
        # by the group that first needs them (see dma_sched)

        # D: depthwise+HS output, [(4b,w), half, g, c4, h]
        D = pers.tile([112, 2, G, 4, H], F16, name="dact")
        # R: channel-major pointwise rhs per kc chunk: [c, half, b4, h, w]
        Rt = [pers.tile([120, 2, 4, H, W], F16, name=f"rt{kc}")
              for kc in range(2)]
        s_sb = [pers.tile([120, NB], F32, name=f"s{kc}") for kc in range(2)]
        g_sb = [pers.tile([120, NB], F32, name=f"gate{mo}") for mo in range(2)]
        sc2 = [pers.tile([120, NB], F32, name=f"sc2{mo}") for mo in range(2)]
        bi2 = [pers.tile([120, NB], F32, name=f"bi2{mo}") for mo in range(2)]

        # input-DMA emission schedule: group -> [(fn, arg), ...], ordered by
        # first-need time with a few groups of lookahead
        LOOK = cfg.get("look", 12)
        dma_sched = {}
        for i in range(2, len(xchunks)):
            dma_sched.setdefault(max(0, xstarts[i] - LOOK), []).append(
                (issue_x, i))
        for c in range(TROT, NTC):
            dma_sched.setdefault((c - TROT) * TCH + TCH - 1, []).append(
                (fill_toep, c))

        # ================= Phase A: depthwise + BN1 + hardswish
        copy_rr = cfg["copy_rr"]
        cp_i = 0
        if True:
            pa = tc.alloc_tile_pool(name="pa", bufs=4)
            dwps = tc.alloc_tile_pool(name="dwps", bufs=cfg["dwbufs"],
                                      space="PSUM")
            t2ps = tc.alloc_tile_pool(name="t2ps", bufs=cfg["t2bufs"],
                                      space="PSUM", side="right")

            stt_eng = ENG[cfg["stt_eng"]]

            def do_group(g):
                ci, co = xmap[g]
                xg = x_ch[ci]
                tg, to = t_sl[(g // TCH) % TROT], g % TCH
                ps = dwps.tile([128, 2, 112], F32, tag="dw")
                n = 0
                for half in (0, 1):
                    for dx in range(KK):
                        # lhsT free (32w, 4b) strides (4,1) merges to (128,1)
                        nc.tensor.matmul(
                            ps[:, half, :],
                            xg[:, co, half, dx:dx + 32, :],
                            tg[:, to, :, dx, :],
                            start=(n == 0), stop=(n == 9))
                        n += 1
                a = pa.tile([112, 2, 112], F16, tag="a")
                nc.scalar.activation(a[:], ps[0:112], AF.Relu)
                m = pa.tile([112, 2, 112], F16, tag="m")
                nc.vector.tensor_scalar(m[:], a[:], 1.0 / 6.0, 1.0,
                                        AL.mult, AL.min)
                dst = D[:, :, g, :, :].rearrange("p half c h -> p half (c h)")
                stt_eng.scalar_tensor_tensor(dst, a[:], 3.0, m[:],
                                             AL.subtract, AL.mult)
                # emit upcoming input DMAs in the order they will be needed
                for fn, arg in dma_sched.get(g, ()):
                    fn(arg)

            red_rr = cfg["red_rr"]
            trash = pers.tile([120, HW], F16, name="trash")
            rd_i = 0

            def do_t2(kc, half):
                nonlocal cp_i, rd_i
                for h0 in range(0, H, 4):
                    tp = t2ps.tile([120, 4, 112], F16, tag="t2")
                    for hi in range(4):
                        nc.tensor.transpose(
                            tp[:, hi, :],
                            D[:, half, 30 * kc:30 * kc + 30, :, h0 + hi],
                            ident[:112, :112])
                    e = copy_rr[cp_i % len(copy_rr)]
                    cp_i += 1
                    cdst = Rt[kc][:, half, :, h0:h0 + 4, :]
                    csrc = tp[:].rearrange("p hh (w b) -> p b hh w", b=4)
                    if e == "s":
                        nc.scalar.copy(cdst, csrc)
                    else:
                        ENG[e].tensor_copy(cdst, csrc)
                # SE partial reduces for this (kc, half) as soon as ready
                for b4 in range(4):
                    scol = s_sb[kc][:, 4 * half + b4:4 * half + b4 + 1]
                    rsrc = Rt[kc][:, half, b4].rearrange("p h w -> p (h w)")
                    e = red_rr[rd_i % len(red_rr)]
                    rd_i += 1
                    if e == "s":
                        nc.scalar.activation(trash[:], rsrc, AF.Identity,
                                             accum_out=scol)
                    else:
                        nc.vector.tensor_reduce(
                            scol, rsrc, mybir.AxisListType.X, AL.add)

            for g in range(30):
                do_group(g)
            do_t2(0, 0)
            do_t2(0, 1)
            for g in range(30, 60):
                do_group(g)
            dwps.release()
            pa.release()

        # ================= Phase C + D, pipelined per image-half
        d_rr = cfg["d_rr"]
        d_i = 0
        rflat = [Rt[kc][:].rearrange("p half b h w -> p (half b h w)")
                 for kc in range(2)]
        # output staging: [120, half, b4, h, w] per mo; 1 DMA per (mo, half)
        ystage = [pers.tile([120, 2, 4, H, W], F16, name=f"ys{mo}")
                  for mo in range(2)]
        ysflat = [ystage[mo][:].rearrange("p half b h w -> p (half b h w)")
                  for mo in range(2)]
        sep = tc.alloc_tile_pool(name="se", bufs=1)
        seps = tc.alloc_tile_pool(name="seps", bufs=1, space="PSUM")
        pdps = None

        def se_chain(half):
            """gate + fused BN2 scale/bias for images 4*half..4*half+4"""
            hs = slice(4 * half, 4 * half + 4)
            ps1 = seps.tile([R, 4], F32, tag="se1")
            for kc in range(2):
                nc.tensor.matmul(ps1[:], se1l_sb[:, kc, :], s_sb[kc][:, hs],
                                 start=(kc == 0), stop=(kc == 1))
            h1 = sep.tile([R, 4], F32, name=f"h1_{half}")
            nc.scalar.activation(h1[:], ps1[:], AF.Relu, bias=se1b_sb)
            for mo in range(2):
                ps2 = seps.tile([120, 4], F32, tag="se2")
                nc.tensor.matmul(ps2[:], se2l_sb[:, mo, :], h1[:],
                                 start=True, stop=True)
                a2 = sep.tile([120, 4], F32, name=f"a2_{half}_{mo}")
                nc.scalar.activation(a2[:], ps2[:], AF.Relu,
                                     bias=se2b3_sb[:, mo:mo + 1])
                m2 = sep.tile([120, 4], F32, name=f"m2_{half}_{mo}")
                nc.vector.tensor_scalar(m2[:], a2[:], 1.0 / 6.0, 1.0,
                                        AL.mult, AL.min)
                nc.vector.scalar_tensor_tensor(g_sb[mo][:, hs], a2[:], 3.0,
                                               m2[:], AL.subtract, AL.mult)
                # scale2 = s2*g ; bias2 = (s2*pw_b)*g + t2  (per image col)
                nc.vector.tensor_tensor(
                    sc2[mo][:, hs], g_sb[mo][:, hs],
                    bn2s_sb[:, mo:mo + 1].to_broadcast((120, 4)), AL.mult)
                tmpb = sep.tile([120, 4], F32, name=f"tb_{half}_{mo}")
                nc.gpsimd.tensor_tensor(
                    tmpb[:], g_sb[mo][:, hs],
                    bn2sb_sb[:, mo:mo + 1].to_broadcast((120, 4)), AL.mult)
                nc.gpsimd.tensor_tensor(
                    bi2[mo][:, hs], tmpb[:],
                    bn2t_sb[:, mo:mo + 1].to_broadcast((120, 4)), AL.add)

        def do_d(half):
            nonlocal d_i
            for mo in range(2):
                for b in range(4 * half, 4 * half + 4):
                    for nt in range(2):
                        off = b * HW + nt * NT
                        ps = pdps.tile([120, NT], F32, tag="pw")
                        for kc in range(2):
                            nc.tensor.matmul(ps[:], pwl_sb[:, kc, mo, :],
                                             rflat[kc][:, off:off + NT],
                                             start=(kc == 0), stop=(kc == 1))
                        o = ysflat[mo][:, off:off + NT]
                        e = d_rr[d_i % len(d_rr)]
                        d_i += 1
                        if e == "s":
                            nc.scalar.activation(o, ps[:], AF.Identity,
                                                 bias=bi2[mo][:, b:b + 1],
                                                 scale=sc2[mo][:, b:b + 1])
                        else:
                            ENG[e].tensor_scalar(o, ps[:], sc2[mo][:, b:b + 1],
                                                 bi2[mo][:, b:b + 1],
                                                 AL.mult, AL.add)
                for bp in range(2):
                    b0 = 4 * half + 2 * bp
                    y_ap = y_p[b0:b0 + 2,
                               mo * 120:(mo + 1) * 120].rearrange(
                        "b c h w -> c b (h w)")
                    nc.sync.dma_start(
                        y_ap,
                        ystage[mo][:, half, 2 * bp:2 * bp + 2].rearrange(
                            "p b h w -> p b (h w)"))

        do_t2(1, 0)
        do_t2(1, 1)
        t2ps.release()
        pdps = tc.alloc_tile_pool(name="pdps", bufs=cfg["pdbufs"],
                                  space="PSUM")
        se_chain(0)
        se_chain(1)
        do_d(0)
        do_d(1)

        pdps.release()
        seps.release()
        sep.release()
        pers.release()
        cst.release()

    nc.compile()
    _BUILD_CACHE[key] = nc
    return nc


# ---------------------------------------------------------------- host prep
def prep_inputs(inputs, cfg_key=None):
    f32, f16 = np.float32, np.float16

    x = np.asarray(inputs["x"], f32)
    dw_w = np.asarray(inputs["dw_w"], f32)      # [C,1,5,5]
    dw_b = np.asarray(inputs["dw_b"], f32)
    bn1_g = np.asarray(inputs["bn1_g"], f32)
    bn1_b = np.asarray(inputs["bn1_b"], f32)
    bn1_m = np.asarray(inputs["bn1_m"], f32)
    bn1_v = np.asarray(inputs["bn1_v"], f32)
    pw_w = np.asarray(inputs["pw_w"], f32)      # [Cout, C]
    pw_b = np.asarray(inputs["pw_b"], f32)
    se_w1 = np.asarray(inputs["se_w1"], f32)    # [R, C]
    se_b1 = np.asarray(inputs["se_b1"], f32)
    se_w2 = np.asarray(inputs["se_w2"], f32)    # [Cout, R]
    se_b2 = np.asarray(inputs["se_b2"], f32)
    bn2_g = np.asarray(inputs["bn2_g"], f32)
    bn2_b = np.asarray(inputs["bn2_b"], f32)
    bn2_m = np.asarray(inputs["bn2_m"], f32)
    bn2_v = np.asarray(inputs["bn2_v"], f32)

    s1 = bn1_g / np.sqrt(bn1_v + EPS)
    t1 = s1 * (dw_b - bn1_m) + bn1_b

    # Compact Toeplitz [XP, G, KK, 56]: block kb rows base_k + 28*c_in + h_in
    # hold s1[ch]*w[ch, h_in-h_out+2, dx] at col 28*c_in + h_out; row
    # base_k+56 holds t1+3 (dx=0 only).  The device expands this to the
    # block-diagonal [121, 112] rhs via two column-offset DMAs per slot.
    hin = np.arange(H)[:, None]
    hout = np.arange(H)[None, :]
    Dh = hin - hout
    mask = np.abs(Dh) <= 2
    dyi = np.clip(Dh + 2, 0, 4)
    k = dw_w[:, 0] * s1[:, None, None]                        # [C, 5, 5]
    band = np.where(mask[None, :, :, None], k[:, dyi, :], 0.0)  # [C,hin,hout,dx]
    tpar = np.zeros((XP, G, KK, 56), f32)
    for kb in range(2):
        base = 64 * kb
        for ci in range(2):
            ch = np.arange(G) * 4 + 2 * kb + ci               # [G]
            col = 28 * ci
            tpar[base + 28 * ci:base + 28 * ci + 28, :, :,
                 col:col + 28] = \
                band[ch].transpose(1, 0, 3, 2)                # [hin, G, dx, hout]
            tpar[base + 56, :, 0, col:col + 28] = \
                (t1[ch] + 3.0)[:, None]
    tpar = tpar.astype(f16)

    # pointwise weights [K=c(120), kc, mo, M=o(120)]
    pwT = pw_w.T                                              # [C, Cout]
    pwl = np.zeros((120, 2, 2, 120), f32)
    for kc in range(2):
        for mo in range(2):
            pwl[:, kc, mo, :] = pwT[kc * 120:(kc + 1) * 120,
                                    mo * 120:(mo + 1) * 120]
    pwl = pwl.astype(f16)

    s2 = bn2_g / np.sqrt(bn2_v + EPS)
    cblob = np.zeros((120, NCB), f32)
    # se1l [120, (kc, r)] = w1T[kc*120+p, r] / HW
    cblob[:, 0:120] = (se_w1.T / HW).reshape(2, 120, R).transpose(
        1, 0, 2).reshape(120, 120)
    cblob[:R, 120] = se_b1
    cblob[:R, 121:361] = se_w2.T.reshape(R, 240)
    cblob[:, 361:363] = (se_b2 + 3.0).reshape(2, 120).T
    cblob[:, 363:365] = s2.reshape(2, 120).T
    cblob[:, 365:367] = (s2 * pw_b).reshape(2, 120).T
    cblob[:, 367:369] = (bn2_b - bn2_m * s2).reshape(2, 120).T

    shared = {
        "tpar": tpar, "pwl": pwl, "cblob": cblob,
        "zer": np.zeros((XP, CFG["tch"], KK, 56), f16),
    }

    # x arena [XP, G, half, 36w, 4b]: rows base_k + 28*c_loc + h hold
    # x[4*half+b4, ch, h, j-2] (zero padded in w); row base_k+56 = 1.0
    x16 = x.astype(f16)
    in_maps = []
    for core in range(N_CORES):
        xc = x16[core * NB:(core + 1) * NB]                   # [NB, C, H, W]
        xh = xc.reshape(2, 4, C, H, W)                        # [half, b4, ...]
        xar = np.zeros((XP, G, 2, 36, 4), f16)
        for kb in range(2):
            base = 64 * kb
            for ci in range(2):
                ch = np.arange(G) * 4 + 2 * kb + ci
                # [half, b4, G, H, W] -> [H, G, half, W, b4]
                xar[base + 28 * ci:base + 28 * ci + 28, :, :, 2:2 + W, :] = \
                    xh[:, :, ch].transpose(3, 2, 0, 4, 1)
            xar[base + 56] = 1.0
        m = dict(shared)
        m["xar"] = xar
        in_maps.append(m)
    return in_maps


def kernel(**inputs):
    nc = build_nc()
    in_maps = prep_inputs(inputs)
    res = run_bass_kernel_spmd(nc, in_maps, list(range(N_CORES)))
    out = np.concatenate(
        [np.asarray(res.results[i]["y"]) for i in range(N_CORES)], axis=0)
    return out.astype(np.float32)


# revision 54
# speedup vs baseline: 2.3105x; 1.0045x over previous
"""Trainium2 Bass kernel for nn_DepthWiseSepConv (depthwise 5x5 + BN+hardswish
+ pointwise 1x1 + squeeze-excite gating + BN), data-parallel over batch on
8 NeuronCores.

Self-contained: hardcodes all shapes from the problem spec.

Per-core layout (B_loc = 8 images per core), all matmul operands fp16:

  - Depthwise conv, operand-swapped Toeplitz form: stationary lhsT = x
    [(2ch x 28h_in) + ones-row, (4b x 28w_out)], moving rhs = per-group
    block-diagonal Toeplitz [(2ch x 28h_in) + bias-row, (2ch x 28h_out)].
    Output lands pixel-major [(32w, 4b), (c4, h)] so only ONE transpose
    stage is needed (to channel-major) before the pointwise conv.  The x
    arena is packed (w-major, b-inner) so the stationary operand's free
    dims merge to a single walrus-legal dimension.  BN1 scale is folded
    into the Toeplitz weights; BN1 bias (+3 for the hardswish trick) rides
    an extra ones-row of x (dx=0 slice only).  5 dx shifts accumulate in
    PSUM.  Channel blocks sit at partition bases 0 and 64 (tile_position
    rule); the first rotation of Toeplitz slots is filled full-pad (zeros
    baked in DRAM), later refills overwrite only the payload columns.
  - Hardswish epilogue: ACT relu (PSUM->f16), DVE min, DVE (a-3)*m -> D.
  - One transpose stage (T2): [(4b,w), (g,c)@h] -> channel-major R tiles.
  - SE: per-image DVE reduces on R, two small matmuls, hardswish; gate and
    BN2 are folded into per-(image,half) ACT scale/bias vectors.
  - Pointwise: [120c x 120o] fp16 matmuls, N=392, single fused epilogue op.
"""

import sys

sys.path.insert(0, "/opt/trn_rl_repo")

import numpy as np

import concourse.bass as bass
import concourse.mybir as mybir
import concourse.tile as tile
from concourse import bacc
from concourse.bass_utils import run_bass_kernel_spmd
from concourse.masks import make_identity

# ---------------------------------------------------------------- constants
N_CORES = 8
B, C, H, W = 64, 240, 28, 28
NB = B // N_CORES           # images per core
KK = 5                      # depthwise kernel size
G = 60                      # groups of 4 channels
Cout = 240
R = 60                      # SE reduction dim
HW = H * W
EPS = 1e-5
NT = 392                    # pointwise free-dim tile (half an image)
XP = 121                    # arena partitions: block0 @0..57, block1 @64..121
NCB = 369                   # packed f32 const-blob columns

CFG = {
    "xchunks": (4, 8, 10, 12, 12, 14),  # group counts per x DMA chunk
    "tch": 4,               # groups per toeplitz rotation slot
    "trot": 4,              # toeplitz rotation depth
    "dwbufs": 5,            # DW psum pool depth
    "t2bufs": 3,            # T2 psum pool depth
    "copy_rr": "vvs",       # T2 copy engine pattern (v=DVE, s=ACT, g=Pool)
    "d_rr": "sv",          # phase D epilogue engine pattern
    "stt_eng": "v",         # hardswish (a-3)*m engine
    "red_rr": "vs",         # SE reduce engines (v=DVE, s=ACT accum)
    "look": 18,             # input DMA lookahead (groups)
    "pdbufs": 6,            # pointwise psum pool depth
}

F16 = mybir.dt.float16
F32 = mybir.dt.float32


# ---------------------------------------------------------------- builder
_BUILD_CACHE = {}


def build_nc(cfg_key=None):
    cfg = dict(CFG)
    if cfg_key is not None:
        cfg.update(cfg_key)
    key = tuple(sorted((k, str(v)) for k, v in cfg.items()))
    if key in _BUILD_CACHE:
        return _BUILD_CACHE[key]

    nc = bacc.Bacc("TRN2", target_bir_lowering=False, debug=False,
                   num_devices=N_CORES)

    xar_p = nc.declare_dram_parameter("xar", [XP, G, 2, 36, 4], F16,
                                      isOutput=False)
    tpar_p = nc.declare_dram_parameter("tpar", [XP, G, KK, 56], F16,
                                       isOutput=False)
    tpad_p = nc.declare_dram_parameter(
        "tpad", [XP, cfg["trot"] * cfg["tch"], 2, KK, 56], F16,
        isOutput=False)
    pwl_p = nc.declare_dram_parameter("pwl", [120, 2, 2, 120], F16,
                                      isOutput=False)
    cblob_p = nc.declare_dram_parameter("cblob", [120, NCB], F32,
                                        isOutput=False)
    y_p = nc.declare_dram_parameter("y", [NB, Cout, H, W], F16, isOutput=True)

    AL = mybir.AluOpType
    AF = mybir.ActivationFunctionType

    ENG = {"v": nc.vector, "s": nc.scalar, "g": nc.gpsimd}

    with tile.TileContext(nc) as tc:
        cst = tc.alloc_tile_pool(name="cst", bufs=1)
        pers = tc.alloc_tile_pool(name="pers", bufs=1)

        # ---- persistent arenas (chunked input DMAs for pipelined start)
        xchunks = list(cfg["xchunks"])
        assert sum(xchunks) == G
        xstarts = [sum(xchunks[:i]) for i in range(len(xchunks))]
        x_ch = [pers.tile([XP, n, 2, 36, 4], F16, name=f"xch{i}")
                for i, n in enumerate(xchunks)]
        xmap = {}
        for i, (s, n) in enumerate(zip(xstarts, xchunks)):
            for j in range(n):
                xmap[s + j] = (i, j)

        # toeplitz rotation slots: [121, TCH, blk, 5, 56]; the matmul rhs
        # view [:, g, :, dx, :] is the block-diagonal [121, 112].  Blk-major
        # keeps refill DMA runs at 560B (no small-transfer penalty).
        TCH, TROT = cfg["tch"], cfg["trot"]
        NTC = G // TCH
        t_sl = [pers.tile([XP, TCH, 2, KK, 56], F16, name=f"tsl{i}")
                for i in range(TROT)]

        def fill_toep(c):
            sl = t_sl[c % TROT]
            s = c * TCH
            if c < TROT:
                # first rotation: full-pad fill, zeros baked in DRAM
                nc.sync.dma_start(sl[:], tpad_p[:, s:s + TCH])
            else:
                # later refills only overwrite the payload block columns
                nc.sync.dma_start(sl[0:64, :, 0], tpar_p[0:64, s:s + TCH])
                nc.sync.dma_start(sl[57:121, :, 1],
                                  tpar_p[57:121, s:s + TCH])

        def issue_x(i):
            nc.sync.dma_start(x_ch[i][:],
                              xar_p[:, xstarts[i]:xstarts[i] + xchunks[i]])

        issue_x(0)
        fill_toep(0)
        issue_x(1)
        for i in range(1, TROT):
            fill_toep(i)

        # ---- constants in SBUF (single packed f32 blob + f16 pwl)
        pwl_sb = cst.tile([120, 2, 2, 120], F16)       # [K=c, kc, mo, M=o]
        nc.sync.dma_start(pwl_sb[:], pwl_p[:])
        cblob = cst.tile([120, NCB], F32)
        nc.sync.dma_start(cblob[:], cblob_p[:])
        se1l_sb = cblob[:, 0:120].rearrange("p (kc r) -> p kc r", kc=2)
        se1b_sb = cblob[0:R, 120:121]
        se2l_sb = cblob[0:R, 121:361].rearrange("p (mo o) -> p mo o", mo=2)
        se2b3_sb = cblob[:, 361:363]
        bn2s_sb = cblob[:, 363:365]
        bn2sb_sb = cblob[:, 365:367]
        bn2t_sb = cblob[:, 367:369]

        ident = cst.tile([128, 128], F16)
        make_identity(nc, ident[:])

        # remaining input DMAs are emitted inside the group loop, ordered
        # by the group that first needs them (see dma_sched)

        # D: depthwise+HS output, [(4b,w), half, g, c4, h]
        D = pers.tile([112, 2, G, 4, H], F16, name="dact")
        # R: channel-major pointwise rhs per kc chunk: [c, half, b4, h, w]
        Rt = [pers.tile([120, 2, 4, H, W], F16, name=f"rt{kc}")
              for kc in range(2)]
        s_sb = [pers.tile([120, NB], F32, name=f"s{kc}") for kc in range(2)]
        g_sb = [pers.tile([120, NB], F32, name=f"gate{mo}") for mo in range(2)]
        sc2 = [pers.tile([120, NB], F32, name=f"sc2{mo}") for mo in range(2)]
        bi2 = [pers.tile([120, NB], F32, name=f"bi2{mo}") for mo in range(2)]

        # input-DMA emission schedule: group -> [(fn, arg), ...], ordered by
        # first-need time with a few groups of lookahead
        LOOK = cfg.get("look", 12)
        dma_sched = {}
        for i in range(2, len(xchunks)):
            dma_sched.setdefault(max(0, xstarts[i] - LOOK), []).append(
                (issue_x, i))
        for c in range(TROT, NTC):
            dma_sched.setdefault((c - TROT) * TCH + TCH - 1, []).append(
                (fill_toep, c))

        # ================= Phase A: depthwise + BN1 + hardswish
        copy_rr = cfg["copy_rr"]
        cp_i = 0
        if True:
            pa = tc.alloc_tile_pool(name="pa", bufs=4)
            dwps = tc.alloc_tile_pool(name="dwps", bufs=cfg["dwbufs"],
                                      space="PSUM")
            t2ps = tc.alloc_tile_pool(name="t2ps", bufs=cfg["t2bufs"],
                                      space="PSUM", side="right")

            stt_eng = ENG[cfg["stt_eng"]]

            def do_group(g):
                ci, co = xmap[g]
                xg = x_ch[ci]
                tg, to = t_sl[(g // TCH) % TROT], g % TCH
                ps = dwps.tile([128, 2, 112], F32, tag="dw")
                n = 0
                for half in (0, 1):
                    for dx in range(KK):
                        # lhsT free (32w, 4b) strides (4,1) merges to (128,1)
                        nc.tensor.matmul(
                            ps[:, half, :],
                            xg[:, co, half, dx:dx + 32, :],
                            tg[:, to, :, dx, :],
                            start=(n == 0), stop=(n == 9))
                        n += 1
                a = pa.tile([112, 2, 112], F16, tag="a")
                nc.scalar.activation(a[:], ps[0:112], AF.Relu)
                m = pa.tile([112, 2, 112], F16, tag="m")
                nc.vector.tensor_scalar(m[:], a[:], 1.0 / 6.0, 1.0,
                                        AL.mult, AL.min)
                dst = D[:, :, g, :, :].rearrange("p half c h -> p half (c h)")
                stt_eng.scalar_tensor_tensor(dst, a[:], 3.0, m[:],
                                             AL.subtract, AL.mult)
                # emit upcoming input DMAs in the order they will be needed
                for fn, arg in dma_sched.get(g, ()):
                    fn(arg)

            red_rr = cfg["red_rr"]
            trash = pers.tile([120, HW], F16, name="trash")
            rd_i = 0

            def do_t2(kc, half):
                nonlocal cp_i, rd_i
                for h0 in range(0, H, 4):
                    tp = t2ps.tile([120, 4, 112], F16, tag="t2")
                    for hi in range(4):
                        nc.tensor.transpose(
                            tp[:, hi, :],
                            D[:, half, 30 * kc:30 * kc + 30, :, h0 + hi],
                            ident[:112, :112])
                    e = copy_rr[cp_i % len(copy_rr)]
                    cp_i += 1
                    cdst = Rt[kc][:, half, :, h0:h0 + 4, :]
                    csrc = tp[:].rearrange("p hh (w b) -> p b hh w", b=4)
                    if e == "s":
                        nc.scalar.copy(cdst, csrc)
                    else:
                        ENG[e].tensor_copy(cdst, csrc)
                # SE partial reduces for this (kc, half) as soon as ready
                for b4 in range(4):
                    scol = s_sb[kc][:, 4 * half + b4:4 * half + b4 + 1]
                    rsrc = Rt[kc][:, half, b4].rearrange("p h w -> p (h w)")
                    e = red_rr[rd_i % len(red_rr)]
                    rd_i += 1
                    if e == "s":
                        nc.scalar.activation(trash[:], rsrc, AF.Identity,
                                             accum_out=scol)
                    else:
                        nc.vector.tensor_reduce(
                            scol, rsrc, mybir.AxisListType.X, AL.add)

            for g in range(30):
                do_group(g)
            do_t2(0, 0)
            do_t2(0, 1)
            for g in range(30, 60):
                do_group(g)
            dwps.release()
            pa.release()

        # ================= Phase C + D, pipelined per image-half
        d_rr = cfg["d_rr"]
        d_i = 0
        rflat = [Rt[kc][:].rearrange("p half b h w -> p (half b h w)")
                 for kc in range(2)]
        # output staging: [120, half, b4, h, w] per mo; 1 DMA per (mo, half)
        ystage = [pers.tile([120, 2, 4, H, W], F16, name=f"ys{mo}")
                  for mo in range(2)]
        ysflat = [ystage[mo][:].rearrange("p half b h w -> p (half b h w)")
                  for mo in range(2)]
        seps = tc.alloc_tile_pool(name="seps", bufs=1, space="PSUM")
        pdps = None

        def se_chain(half):
            """gate + fused BN2 scale/bias for images 4*half..4*half+4"""
            hs = slice(4 * half, 4 * half + 4)
            ps1 = seps.tile([R, 4], F32, tag="se1")
            for kc in range(2):
                nc.tensor.matmul(ps1[:], se1l_sb[:, kc, :], s_sb[kc][:, hs],
                                 start=(kc == 0), stop=(kc == 1))
            h1 = pers.tile([R, 4], F32, name=f"h1_{half}")
            nc.scalar.activation(h1[:], ps1[:], AF.Relu, bias=se1b_sb)
            for mo in range(2):
                ps2 = seps.tile([120, 4], F32, tag="se2")
                nc.tensor.matmul(ps2[:], se2l_sb[:, mo, :], h1[:],
                                 start=True, stop=True)
                a2 = pers.tile([120, 4], F32, name=f"a2_{half}_{mo}")
                nc.scalar.activation(a2[:], ps2[:], AF.Relu,
                                     bias=se2b3_sb[:, mo:mo + 1])
                m2 = pers.tile([120, 4], F32, name=f"m2_{half}_{mo}")
                nc.vector.tensor_scalar(m2[:], a2[:], 1.0 / 6.0, 1.0,
                                        AL.mult, AL.min)
                nc.vector.scalar_tensor_tensor(g_sb[mo][:, hs], a2[:], 3.0,
                                               m2[:], AL.subtract, AL.mult)
                # scale2 = s2*g ; bias2 = (s2*pw_b)*g + t2  (per image col)
                nc.vector.tensor_tensor(
                    sc2[mo][:, hs], g_sb[mo][:, hs],
                    bn2s_sb[:, mo:mo + 1].to_broadcast((120, 4)), AL.mult)
                tmpb = pers.tile([120, 4], F32, name=f"tb_{half}_{mo}")
                nc.gpsimd.tensor_tensor(
                    tmpb[:], g_sb[mo][:, hs],
                    bn2sb_sb[:, mo:mo + 1].to_broadcast((120, 4)), AL.mult)
                nc.gpsimd.tensor_tensor(
                    bi2[mo][:, hs], tmpb[:],
                    bn2t_sb[:, mo:mo + 1].to_broadcast((120, 4)), AL.add)

        def do_d(half):
            nonlocal d_i
            for mo in range(2):
                for b in range(4 * half, 4 * half + 4):
                    for nt in range(2):
                        off = b * HW + nt * NT
                        ps = pdps.tile([120, NT], F32, tag="pw")
                        for kc in range(2):
                            nc.tensor.matmul(ps[:], pwl_sb[:, kc, mo, :],
                                             rflat[kc][:, off:off + NT],
                                             start=(kc == 0), stop=(kc == 1))
                        o = ysflat[mo][:, off:off + NT]
                        e = d_rr[d_i % len(d_rr)]
                        d_i += 1
                        if e == "s":
                            nc.scalar.activation(o, ps[:], AF.Identity,
                                                 bias=bi2[mo][:, b:b + 1],
                                                 scale=sc2[mo][:, b:b + 1])
                        else:
                            ENG[e].tensor_scalar(o, ps[:], sc2[mo][:, b:b + 1],
                                                 bi2[mo][:, b:b + 1],
                                                 AL.mult, AL.add)
                nyd = 4 if half == 1 else 2
                for bp in range(nyd):
                    w0 = 4 // nyd
                    b0 = 4 * half + w0 * bp
                    y_ap = y_p[b0:b0 + w0,
                               mo * 120:(mo + 1) * 120].rearrange(
                        "b c h w -> c b (h w)")
                    nc.sync.dma_start(
                        y_ap,
                        ystage[mo][:, half, w0 * bp:w0 * bp + w0].rearrange(
                            "p b h w -> p b (h w)"))

        if cfg.get("d_interleave"):
            pdps = tc.alloc_tile_pool(name="pdps", bufs=cfg["pdbufs"],
                                      space="PSUM")
            do_t2(1, 0)
            se_chain(0)
            do_d(0)
            do_t2(1, 1)
            se_chain(1)
            do_d(1)
            t2ps.release()
        else:
            do_t2(1, 0)
            do_t2(1, 1)
            t2ps.release()
            pdps = tc.alloc_tile_pool(name="pdps", bufs=cfg["pdbufs"],
                                      space="PSUM")
            se_chain(0)
            se_chain(1)
            do_d(0)
            do_d(1)

        pdps.release()
        seps.release()
        pers.release()
        cst.release()

    nc.compile()
    _BUILD_CACHE[key] = nc
    return nc


# ---------------------------------------------------------------- host prep
def prep_inputs(inputs, cfg_key=None):
    f32, f16 = np.float32, np.float16

    x = np.asarray(inputs["x"], f32)
    dw_w = np.asarray(inputs["dw_w"], f32)      # [C,1,5,5]
    dw_b = np.asarray(inputs["dw_b"], f32)
    bn1_g = np.asarray(inputs["bn1_g"], f32)
    bn1_b = np.asarray(inputs["bn1_b"], f32)
    bn1_m = np.asarray(inputs["bn1_m"], f32)
    bn1_v = np.asarray(inputs["bn1_v"], f32)
    pw_w = np.asarray(inputs["pw_w"], f32)      # [Cout, C]
    pw_b = np.asarray(inputs["pw_b"], f32)
    se_w1 = np.asarray(inputs["se_w1"], f32)    # [R, C]
    se_b1 = np.asarray(inputs["se_b1"], f32)
    se_w2 = np.asarray(inputs["se_w2"], f32)    # [Cout, R]
    se_b2 = np.asarray(inputs["se_b2"], f32)
    bn2_g = np.asarray(inputs["bn2_g"], f32)
    bn2_b = np.asarray(inputs["bn2_b"], f32)
    bn2_m = np.asarray(inputs["bn2_m"], f32)
    bn2_v = np.asarray(inputs["bn2_v"], f32)

    s1 = bn1_g / np.sqrt(bn1_v + EPS)
    t1 = s1 * (dw_b - bn1_m) + bn1_b

    # Compact Toeplitz [XP, G, KK, 56]: block kb rows base_k + 28*c_in + h_in
    # hold s1[ch]*w[ch, h_in-h_out+2, dx] at col 28*c_in + h_out; row
    # base_k+56 holds t1+3 (dx=0 only).  The device expands this to the
    # block-diagonal [121, 112] rhs via two column-offset DMAs per slot.
    hin = np.arange(H)[:, None]
    hout = np.arange(H)[None, :]
    Dh = hin - hout
    mask = np.abs(Dh) <= 2
    dyi = np.clip(Dh + 2, 0, 4)
    k = dw_w[:, 0] * s1[:, None, None]                        # [C, 5, 5]
    band = np.where(mask[None, :, :, None], k[:, dyi, :], 0.0)  # [C,hin,hout,dx]
    tpar = np.zeros((XP, G, KK, 56), f32)
    for kb in range(2):
        base = 64 * kb
        for ci in range(2):
            ch = np.arange(G) * 4 + 2 * kb + ci               # [G]
            col = 28 * ci
            tpar[base + 28 * ci:base + 28 * ci + 28, :, :,
                 col:col + 28] = \
                band[ch].transpose(1, 0, 3, 2)                # [hin, G, dx, hout]
            tpar[base + 56, :, 0, col:col + 28] = \
                (t1[ch] + 3.0)[:, None]
    tpar = tpar.astype(f16)

    # pointwise weights [K=c(120), kc, mo, M=o(120)]
    pwT = pw_w.T                                              # [C, Cout]
    pwl = np.zeros((120, 2, 2, 120), f32)
    for kc in range(2):
        for mo in range(2):
            pwl[:, kc, mo, :] = pwT[kc * 120:(kc + 1) * 120,
                                    mo * 120:(mo + 1) * 120]
    pwl = pwl.astype(f16)

    s2 = bn2_g / np.sqrt(bn2_v + EPS)
    cblob = np.zeros((120, NCB), f32)
    # se1l [120, (kc, r)] = w1T[kc*120+p, r] / HW
    cblob[:, 0:120] = (se_w1.T / HW).reshape(2, 120, R).transpose(
        1, 0, 2).reshape(120, 120)
    cblob[:R, 120] = se_b1
    cblob[:R, 121:361] = se_w2.T.reshape(R, 240)
    cblob[:, 361:363] = (se_b2 + 3.0).reshape(2, 120).T
    cblob[:, 363:365] = s2.reshape(2, 120).T
    cblob[:, 365:367] = (s2 * pw_b).reshape(2, 120).T
    cblob[:, 367:369] = (bn2_b - bn2_m * s2).reshape(2, 120).T

    npad = CFG["trot"] * CFG["tch"]
    tpad = np.zeros((XP, npad, 2, KK, 56), f16)
    tpad[0:64, :, 0] = tpar[0:64, 0:npad].transpose(0, 1, 2, 3)
    tpad[57:121, :, 1] = tpar[57:121, 0:npad]
    shared = {
        "tpar": tpar, "pwl": pwl, "cblob": cblob, "tpad": tpad,
    }

    # x arena [XP, G, half, 36w, 4b]: rows base_k + 28*c_loc + h hold
    # x[4*half+b4, ch, h, j-2] (zero padded in w); row base_k+56 = 1.0
    x16 = x.astype(f16)
    in_maps = []
    for core in range(N_CORES):
        xc = x16[core * NB:(core + 1) * NB]                   # [NB, C, H, W]
        xh = xc.reshape(2, 4, C, H, W)                        # [half, b4, ...]
        xar = np.zeros((XP, G, 2, 36, 4), f16)
        for kb in range(2):
            base = 64 * kb
            for ci in range(2):
                ch = np.arange(G) * 4 + 2 * kb + ci
                # [half, b4, G, H, W] -> [H, G, half, W, b4]
                xar[base + 28 * ci:base + 28 * ci + 28, :, :, 2:2 + W, :] = \
                    xh[:, :, ch].transpose(3, 2, 0, 4, 1)
            xar[base + 56] = 1.0
        m = dict(shared)
        m["xar"] = xar
        in_maps.append(m)
    return in_maps


def kernel(**inputs):
    nc = build_nc()
    in_maps = prep_inputs(inputs)
    res = run_bass_kernel_spmd(nc, in_maps, list(range(N_CORES)))
    out = np.concatenate(
        [np.asarray(res.results[i]["y"]) for i in range(N_CORES)], axis=0)
    return out.astype(np.float32)
